# revision 46
# baseline (speedup 1.0000x reference)
"""Trainium2 Bass kernel for nn_Attention_85564338471023.

Multi-head self-attention (B=16, N=1024 tokens, C=512, 8 heads x d=64) with
qkv projection, softmax attention, output projection and residual.

Sharding: pure data-parallel over batch -- 2 batch elements per NeuronCore,
no collectives. Host pre-transposes x (channels-on-partitions) and reorders
w_qkv columns so heads come in pairs that share 128-partition tiles.

Pipeline (204us baseline -> ~140us):
  * Head-pair concurrency on the PE: the two heads' score matmuls (K=64
    stationaries at row offsets 0/64) and AV/denominator matmuls (M=64
    outputs at partition offsets 0/64) are emitted adjacently so the PE
    row-/col-tiles them into disjoint quadrants and streams them
    concurrently. Score tiles must stay SEPARATE PSUM tiles per head --
    merging both heads into one [128,1024] tile serializes the pair.
  * PSUM (8 banks) split so all phases coexist: scores 5x[128,512]
    (5-slot pool decouples slot recycling from the exp drain order),
    res+s accumulators 2, projection scratch 1.
  * Single global software pipeline over the head-pair stream (across
    batch boundaries): pair g's scores+exp overlap pair g-1's AV
    matmuls; qkv/v projections of batch i+1 and the out-projection of
    batch i-1 are queued as filler chunks, one per step, placed BEFORE
    the score matmuls that wait on exp PSUM-slot releases so the
    in-order PE queue never stalls on ready work.
  * exp(x/8) on ScalarE with fused scale, no max-subtraction (scores are
    ~N(0,1)); softmax denominator via ones-stationary matmuls sharing
    the AV accumulation; 1/s via DVE reciprocal_approx_fast.
  * bf16 everywhere except the q/k tiles and score matmuls (f32r): x,
    w_qkv, w_v, w_out, AV operands and rt. All bf16 stationaries are
    64-col (split col-tiled halves for the projections) -- LDWEIGHTS
    fully hides under the streams and the 128-col fast-weight-load
    corruption path is never engaged. Final rel err ~3e-4.
"""

from collections import deque
from contextlib import ExitStack

import ml_dtypes
import numpy as np

import concourse.bacc as bacc
import concourse.bass as bass
import concourse.tile as tile
from concourse import mybir
from concourse.bass_utils import run_bass_kernel_spmd  # noqa: F401 (fallback path)

N_CORES = 8
B, HH, WW, C = 16, 32, 32, 512
N = HH * WW            # 1024 tokens
NH, DH = 8, 64
SCALE = DH ** -0.5     # 0.125
BL = B // N_CORES      # 2 batch elements per core
P = 128
F32 = mybir.dt.float32
F32R = mybir.dt.float32r
MM_DT = F32R
AV_DT = mybir.dt.bfloat16


def build_program(repeat=1) -> bass.Bass:
    inner, hw_loop = repeat if isinstance(repeat, tuple) else (repeat, None)
    nc = bacc.Bacc("TRN2", target_bir_lowering=False, debug=False)

    xT_d = nc.dram_tensor("xT", [BL, C, N], AV_DT, kind="ExternalInput")
    rs_d = nc.dram_tensor("resid", [BL, N, C], F32, kind="ExternalInput")
    wqk_d = nc.dram_tensor("wqk", [C, 1024], AV_DT, kind="ExternalInput")
    bqk_d = nc.dram_tensor("bqk", [1024], F32, kind="ExternalInput")
    wv_d = nc.dram_tensor("wv", [C, 512], AV_DT, kind="ExternalInput")
    wout_d = nc.dram_tensor("wout", [C, 512], AV_DT, kind="ExternalInput")
    out_d = nc.dram_tensor("out", [BL, N, C], F32, kind="ExternalOutput")

    with tile.TileContext(nc) as tc, ExitStack() as ctx:
        consts = ctx.enter_context(tc.tile_pool(name="consts", bufs=1))
        wpool = ctx.enter_context(tc.tile_pool(name="w", bufs=1))
        xt_pool = ctx.enter_context(tc.tile_pool(name="xt", bufs=8))
        qk_pool = ctx.enter_context(tc.tile_pool(name="qk", bufs=12))
        v_pool = ctx.enter_context(tc.tile_pool(name="v", bufs=16))
        ex_pool = ctx.enter_context(tc.tile_pool(name="ex", bufs=48))
        rt_pool = ctx.enter_context(tc.tile_pool(name="rt", bufs=18))
        misc = ctx.enter_context(tc.tile_pool(name="misc", bufs=2))
        # PSUM: 8 banks. scores 5x[128,512] (separate tiles per head and
        # x-half -- merging heads into one tile serializes the score
        # matmuls); res+s accumulators = 2; projections = 1.
        psb = ctx.enter_context(tc.tile_pool(name="psb", bufs=5, space="PSUM"))
        pst = ctx.enter_context(tc.tile_pool(name="pst", bufs=1, space="PSUM"))
        psp = ctx.enter_context(tc.tile_pool(name="psp", bufs=1, space="PSUM"))

        ones = consts.tile([P, 64], AV_DT, tag="ones")
        nc.vector.memset(ones[:], 1.0)
        bqk_sb = consts.tile([P, 8], F32, tag="bqk")
        nc.sync.dma_start(out=bqk_sb[:], in_=bqk_d.ap().rearrange("(t p) -> p t", p=P))

        wqk_sb, wv_sb, wout_sb = [], [], []
        for kc in range(4):
            t = wpool.tile([P, 1024], AV_DT, tag=f"wqk{kc}")
            nc.sync.dma_start(out=t[:], in_=wqk_d.ap()[kc * P:(kc + 1) * P, :])
            wqk_sb.append(t)
        for kc in range(4):
            t = wpool.tile([P, 512], AV_DT, tag=f"wv{kc}")
            nc.sync.dma_start(out=t[:], in_=wv_d.ap()[kc * P:(kc + 1) * P, :])
            wv_sb.append(t)
            t = wpool.tile([P, 512], AV_DT, tag=f"wout{kc}")
            nc.sync.dma_start(out=t[:], in_=wout_d.ap()[kc * P:(kc + 1) * P, :])
            wout_sb.append(t)

        # per-live-batch state, keyed by linear batch index i
        xt = {}      # i -> [4 tiles]
        qk = {}      # (i, jt) -> tile
        vo = {}      # (i, yt) -> tile
        ex = {}      # (i, p, yt, e, xs) -> tile
        rt = {}      # (i, p, xs) -> tile
        acc = {}     # (i, p, xs) -> (res_ps, s_ps)

        def emit_xt_dma(i):
            b = i % BL
            xt[i] = []
            for kc in range(4):
                t = xt_pool.tile([P, N], AV_DT)
                nc.sync.dma_start(out=t[:], in_=xT_d.ap()[b, kc * P:(kc + 1) * P, :])
                xt[i].append(t)

        def emit_qk_chunk(i, jt, xs):
            # bf16 stationaries split into 64-col halves: avoids the
            # 128-col fast-weight-load path, halves+hides LDWEIGHTS, and
            # the two halves run col-tiled concurrently.
            ps = psp.tile([P, 512], F32, tag="p")
            for kc in range(4):
                for e2 in range(2):
                    nc.tensor.matmul(
                        ps[e2 * 64:(e2 + 1) * 64, :],
                        wqk_sb[kc][:, jt * P + e2 * 64:jt * P + (e2 + 1) * 64],
                        xt[i][kc][:, xs * 512:(xs + 1) * 512],
                        start=(kc == 0), stop=(kc == 3), skip_group_check=True,
                    )
            if xs == 0:
                qk[(i, jt)] = qk_pool.tile([P, N], MM_DT, tag="qk",
                                           name=f"qk_{i}_{jt}")
            nc.vector.tensor_scalar(
                out=qk[(i, jt)][:, xs * 512:(xs + 1) * 512], in0=ps[:],
                scalar1=bqk_sb[:, jt:jt + 1], scalar2=None,
                op0=mybir.AluOpType.add,
            )

        def emit_v_chunk(i, yt):
            ps = psp.tile([P, 512], F32, tag="p")
            for kc in range(4):
                for e2 in range(2):
                    nc.tensor.matmul(
                        ps[e2 * 64:(e2 + 1) * 64, :],
                        xt[i][kc][:, yt * P + e2 * 64:yt * P + (e2 + 1) * 64],
                        wv_sb[kc][:],
                        start=(kc == 0), stop=(kc == 3), skip_group_check=True,
                    )
            t = v_pool.tile([P, 512], AV_DT)
            nc.vector.tensor_copy(t[:], ps[:])
            vo[(i, yt)] = t

        # Scores for one (pair, yt) step: 4 PSUM tiles [128,512] keyed
        # (e, xs), allocated in the order (e0,x0),(e1,x0),(e0,x1),(e1,x1)
        # to match the exp drain order; the matmul pairs (e0,x) + (e1,x)
        # are adjacent so the PE row-tiles them concurrently (K=64 heads
        # at row offsets 0 / 64).
        def emit_scores(i, p, yt):
            kk = qk[(i, p)]
            qq = qk[(i, 4 + p)]
            sc = {}
            for xs in range(2):
                for e in range(2):
                    sc[(e, xs)] = psb.tile([P, 512], F32, tag="big",
                                           name=f"sc_{e}_{xs}")
            for xs in range(2):
                for e in range(2):
                    nc.tensor.matmul(
                        sc[(e, xs)][:],
                        kk[e * 64:(e + 1) * 64, yt * P:(yt + 1) * P],
                        qq[e * 64:(e + 1) * 64, xs * 512:(xs + 1) * 512],
                        start=True, stop=True,
                    )
            for xs in range(2):
                for e in range(2):
                    t = ex_pool.tile([P, 512], AV_DT, tag="ex",
                                     name=f"ex_{e}_{xs}")
                    nc.scalar.activation(
                        out=t[:], in_=sc[(e, xs)][:],
                        func=mybir.ActivationFunctionType.Exp, scale=SCALE,
                    )
                    ex[(i, p, yt, e, xs)] = t

        # AV chunks for a (pair, xs) group are emitted in y-order
        # 1,0,3,2,5,4,7,6 (so the group-opening chunk never lands right
        # behind the divide that frees its PSUM slot): accumulation-group
        # flags go on the first/last EMITTED chunk.
        def emit_av_chunk(i, p, xs, yt):
            if yt == 1:
                res_ps = pst.tile([P, 512], F32, tag="res")
                s_ps = pst.tile([P, 512], F32, tag="s")
                acc[(i, p, xs)] = (res_ps, s_ps)
            res_ps, s_ps = acc[(i, p, xs)]
            first, last = yt == 1, yt == 6
            exs = [ex[(i, p, yt, e, xs)][:] for e in range(2)]
            for e in range(2):
                h = 2 * p + e
                nc.tensor.matmul(
                    res_ps[e * 64:(e + 1) * 64, :],
                    vo[(i, yt)][:, h * 64:(h + 1) * 64],
                    exs[e],
                    start=first, stop=last, skip_group_check=True,
                )
            for e in range(2):
                nc.tensor.matmul(
                    s_ps[e * 64:(e + 1) * 64, :],
                    ones[:],
                    exs[e],
                    start=first, stop=last, skip_group_check=True,
                )

        def emit_divide(i, p, xs):
            res_ps, s_ps = acc.pop((i, p, xs))
            rec = misc.tile([P, 512], F32, tag="prc")
            nc.vector.reciprocal_approx_fast(out=rec[:], in_=s_ps[:])
            t = rt_pool.tile([P, 512], AV_DT)
            nc.vector.tensor_tensor(
                out=t[:], in0=res_ps[:], in1=rec[:],
                op=mybir.AluOpType.mult,
            )
            rt[(i, p, xs)] = t
            if xs == 1:
                for yy in range(8):
                    for e in range(2):
                        for xx in range(2):
                            del ex[(i, p, yy, e, xx)]
                if p == 3:
                    for yy in range(8):
                        del vo[(i, yy)]

        def emit_out_chunk(i, nt):
            b = i % BL
            xs, sub = divmod(nt, 4)
            ps = psp.tile([P, 512], F32, tag="p")
            # rt stationaries split into 64-col halves: avoids the 128-col
            # bf16 fast-weight-load corruption and the two halves run
            # col-tiled concurrently (out partitions 0-63 / 64-127).
            for p in range(4):
                for e in range(2):
                    nc.tensor.matmul(
                        ps[e * 64:(e + 1) * 64, :],
                        rt[(i, p, xs)][:, sub * P + e * 64:sub * P + (e + 1) * 64],
                        wout_sb[p][:],
                        start=(p == 0), stop=(p == 3), skip_group_check=True,
                    )
            rs = misc.tile([P, 512], F32, tag="rs")
            nc.sync.dma_start(out=rs[:], in_=rs_d.ap()[b, nt * P:(nt + 1) * P, :])
            ob = misc.tile([P, 512], F32, tag="ob")
            nc.vector.tensor_tensor(
                out=ob[:], in0=ps[:], in1=rs[:], op=mybir.AluOpType.add,
            )
            nc.sync.dma_start(out=out_d.ap()[b, nt * P:(nt + 1) * P, :], in_=ob[:])
            if nt == 7:
                for p in range(4):
                    for xs2 in range(2):
                        del rt[(i, p, xs2)]

        def _batches():
            nb = inner * BL
            filler = deque()

            def pump(k):
                for _ in range(k):
                    if not filler:
                        return
                    filler.popleft()()

            # prologue: first batch's input + projections emitted directly
            emit_xt_dma(0)
            for jt in range(8):
                for xs in range(2):
                    emit_qk_chunk(0, jt, xs)
            for yt in range(8):
                emit_v_chunk(0, yt)
            if nb > 1:
                emit_xt_dma(1)

            def queue_batch_work(i):
                # Fillers consumed during batch i's 32 steps (1 per step):
                # projections for batch i+1 interleaved so qk tile slot
                # releases (one pair's kk+qq free every 8 steps) line up,
                # then the out-projection of batch i-1 (whose rt tiles
                # finish during this batch's pair-0 window).
                if i + 1 < nb:
                    if i + 2 < nb:
                        emit_xt_dma(i + 2)
                    def qkc(jt):
                        return [(lambda jt=jt, xs=xs: emit_qk_chunk(i + 1, jt, xs))
                                for xs in range(2)]
                    def vc(y0, y1):
                        return [(lambda yt=yt: emit_v_chunk(i + 1, yt))
                                for yt in range(y0, y1)]
                    for c in (qkc(0) + qkc(1) + qkc(2) + qkc(3) + vc(0, 4)
                              + qkc(4) + qkc(5) + vc(4, 8) + qkc(6) + qkc(7)):
                        filler.append(c)

            # Global pair stream: pair g's scores+exp overlap pair (g-1)'s
            # AV matmuls, across batch boundaries. Per-step PE order
            # [filler, AV, scores] keeps ready work ahead of the score
            # matmuls that wait on exp slot releases.
            G = 4 * nb
            for g in range(G):
                i, p = divmod(g, 4)
                if p == 0:
                    queue_batch_work(i)
                if p == 1 and i >= 1:
                    # batch i-1's rt tiles are complete once pair 0's AV
                    # divides have been emitted (during pair 0's steps)
                    filler.extend(
                        (lambda nt=nt: emit_out_chunk(i - 1, nt))
                        for nt in range(8))
                for yt in range(8):
                    pump(1)
                    if g >= 1:
                        qi, qp = divmod(g - 1, 4)
                        xs, k = divmod(yt, 4)
                        emit_av_chunk(qi, qp, xs, 2 * k + 1)
                        emit_av_chunk(qi, qp, xs, 2 * k)
                        if k == 3:
                            emit_divide(qi, qp, xs)
                    emit_scores(i, p, yt)
            # epilogue: drain the last pair's AV + out-projection
            qi, qp = nb - 1, 3
            for xs in range(2):
                for k in range(4):
                    emit_av_chunk(qi, qp, xs, 2 * k + 1)
                    emit_av_chunk(qi, qp, xs, 2 * k)
                emit_divide(qi, qp, xs)
            for nt in range(8):
                emit_out_chunk(nb - 1, nt)
            pump(len(filler))

        if hw_loop:
            with tc.For_i(0, hw_loop):
                _batches()
        else:
            _batches()
    nc.compile()
    return nc


def host_prep(ft, w_qkv, b_qkv, w_out, b_out):
    ft = np.asarray(ft, dtype=np.float32)
    w_qkv = np.asarray(w_qkv, dtype=np.float32)
    b_qkv = np.asarray(b_qkv, dtype=np.float32)
    w_out = np.asarray(w_out, dtype=np.float32)
    b_out = np.asarray(b_out, dtype=np.float32)

    x = ft.reshape(B, N, C)
    xT = np.ascontiguousarray(x.transpose(0, 2, 1))

    w_qk_re = np.empty((C, 1024), np.float32)
    b_qk_re = np.empty((1024,), np.float32)
    w_v_re = np.empty((C, 512), np.float32)
    for p in range(4):
        hA, hB = 2 * p, 2 * p + 1
        w_qk_re[:, p * 128:p * 128 + 64] = w_qkv[:, hA * 192 + 64:hA * 192 + 128]
        w_qk_re[:, p * 128 + 64:p * 128 + 128] = w_qkv[:, hB * 192 + 64:hB * 192 + 128]
        b_qk_re[p * 128:p * 128 + 64] = b_qkv[hA * 192 + 64:hA * 192 + 128]
        b_qk_re[p * 128 + 64:p * 128 + 128] = b_qkv[hB * 192 + 64:hB * 192 + 128]
        w_qk_re[:, 512 + p * 128:512 + p * 128 + 64] = w_qkv[:, hA * 192:hA * 192 + 64]
        w_qk_re[:, 512 + p * 128 + 64:512 + p * 128 + 128] = w_qkv[:, hB * 192:hB * 192 + 64]
        b_qk_re[512 + p * 128:512 + p * 128 + 64] = b_qkv[hA * 192:hA * 192 + 64]
        b_qk_re[512 + p * 128 + 64:512 + p * 128 + 128] = b_qkv[hB * 192:hB * 192 + 64]
        w_v_re[:, p * 128:p * 128 + 64] = w_qkv[:, hA * 192 + 128:hA * 192 + 192]
        w_v_re[:, p * 128 + 64:p * 128 + 128] = w_qkv[:, hB * 192 + 128:hB * 192 + 192]

    b_v_nat = np.empty((512,), np.float32)
    for h in range(NH):
        b_v_nat[h * 64:(h + 1) * 64] = b_qkv[h * 192 + 128:h * 192 + 192]
    resid = x + b_out[None, None, :] + (b_v_nat @ w_out)[None, None, :]
    resid = np.ascontiguousarray(resid, dtype=np.float32)
    return xT, resid, w_qk_re, b_qk_re, w_v_re, np.ascontiguousarray(w_out)


_NC_CACHE = {}


def get_program(repeat: int = 1) -> bass.Bass:
    if repeat not in _NC_CACHE:
        _NC_CACHE[repeat] = build_program(repeat)
    return _NC_CACHE[repeat]


def make_in_maps(ft, w_qkv, b_qkv, w_out, b_out):
    xT, resid, w_qk_re, b_qk_re, w_v_re, w_out_c = host_prep(
        ft, w_qkv, b_qkv, w_out, b_out)
    in_maps = []
    for core in range(N_CORES):
        sl = slice(core * BL, (core + 1) * BL)
        in_maps.append({
            "xT": np.ascontiguousarray(xT[sl].astype(ml_dtypes.bfloat16)),
            "resid": np.ascontiguousarray(resid[sl]),
            "wqk": np.ascontiguousarray(w_qk_re.astype(ml_dtypes.bfloat16)),
            "bqk": b_qk_re,
            "wv": np.ascontiguousarray(w_v_re.astype(ml_dtypes.bfloat16)),
            "wout": np.ascontiguousarray(w_out_c.astype(ml_dtypes.bfloat16)),
        })
    return in_maps


_RUNNER_CACHE = {}


def make_runner(repeat: int = 1):
    """Build (once) a persistent jitted executor for the bass program.

    Returns run(in_maps) -> list[dict[name, np.ndarray]] per core. Keeping
    the jitted callable alive means repeat calls skip lowering/compile and
    are pure dispatch+execute.
    """
    if repeat in _RUNNER_CACHE:
        return _RUNNER_CACHE[repeat]

    import jax
    from jax.experimental.shard_map import shard_map
    from jax.sharding import Mesh, PartitionSpec
    from concourse import mybir as _mb
    from concourse import bass2jax

    bass2jax.install_neuronx_cc_hook()
    nc = get_program(repeat)

    partition_name = nc.partition_id_tensor.name if nc.partition_id_tensor else None
    in_names, out_names, out_avals, zero_shapes = [], [], [], []
    for alloc in nc.m.functions[0].allocations:
        if not isinstance(alloc, _mb.MemoryLocationSet):
            continue
        name = alloc.memorylocations[0].name
        if alloc.kind == "ExternalInput":
            if name != partition_name:
                in_names.append(name)
        elif alloc.kind == "ExternalOutput":
            np_dt = _mb.dt.np(alloc.dtype)
            out_names.append(name)
            out_avals.append(jax.core.ShapedArray(tuple(alloc.tensor_shape), np_dt))
            zero_shapes.append((tuple(alloc.tensor_shape), np_dt))
    n_params = len(in_names)
    all_in_names = list(in_names) + list(out_names)
    if partition_name is not None:
        all_in_names.append(partition_name)

    def _body(*args):
        operands = list(args)
        if partition_name is not None:
            operands.append(bass2jax.partition_id_tensor())
        outs = bass2jax._bass_exec_p.bind(
            *operands,
            out_avals=tuple(out_avals),
            in_names=tuple(all_in_names),
            out_names=tuple(out_names),
            lowering_input_output_aliases=(),
            sim_require_finite=True,
            sim_require_nnan=True,
            nc=nc,
        )
        return tuple(outs)

    devices = jax.devices()[:N_CORES]
    mesh = Mesh(np.asarray(devices), ("core",))
    n_outs = len(out_names)
    sharded = jax.jit(
        shard_map(_body, mesh=mesh,
                  in_specs=(PartitionSpec("core"),) * (n_params + n_outs),
                  out_specs=(PartitionSpec("core"),) * n_outs,
                  check_rep=False),
        keep_unused=True,
    )

    def run(in_maps):
        concat_in = [
            np.concatenate([np.asarray(m[name]) for m in in_maps], axis=0)
            for name in in_names
        ]
        zeros = [np.zeros((N_CORES * s[0], *s[1:]), dt) for s, dt in zero_shapes]
        out_arrs = sharded(*concat_in, *zeros)
        return [
            {name: np.asarray(out_arrs[i]).reshape(N_CORES, *out_avals[i].shape)[c]
             for i, name in enumerate(out_names)}
            for c in range(N_CORES)
        ]

    def make_chained(k):
        def _chain(*args):
            ins = list(args[:n_params])
            bufs = list(args[n_params:])
            for _ in range(k):
                bufs = list(_body(*ins, *bufs))
            return tuple(bufs)
        return jax.jit(
            shard_map(_chain, mesh=mesh,
                      in_specs=(PartitionSpec("core"),) * (n_params + n_outs),
                      out_specs=(PartitionSpec("core"),) * n_outs,
                      check_rep=False),
            keep_unused=True,
        )

    run.sharded = sharded
    run.in_names = in_names
    run.zero_shapes = zero_shapes
    run.make_chained = make_chained
    run.mesh = mesh
    _RUNNER_CACHE[repeat] = run
    return run


def kernel(ft, w_qkv, b_qkv, w_out, b_out):
    run = make_runner()
    in_maps = make_in_maps(ft, w_qkv, b_qkv, w_out, b_out)
    results = run(in_maps)
    out = np.concatenate([r["out"] for r in results], axis=0)
    return out.reshape(B, HH, WW, C).astype(np.float32)


# revision 48
# speedup vs baseline: 1.0762x; 1.0762x over previous
"""Trainium2 Bass kernel for nn_Attention_85564338471023.

Multi-head self-attention (B=16, N=1024 tokens, C=512, 8 heads x d=64) with
qkv projection, softmax attention, output projection and residual.

Sharding: pure data-parallel over batch -- 2 batch elements per NeuronCore,
no collectives. Host pre-transposes x (channels-on-partitions) and reorders
w_qkv columns so heads come in pairs that share 128-partition tiles.

Pipeline (204us baseline -> ~140us):
  * Head-pair concurrency on the PE: the two heads' score matmuls (K=64
    stationaries at row offsets 0/64) and AV/denominator matmuls (M=64
    outputs at partition offsets 0/64) are emitted adjacently so the PE
    row-/col-tiles them into disjoint quadrants and streams them
    concurrently. Score tiles must stay SEPARATE PSUM tiles per head --
    merging both heads into one [128,1024] tile serializes the pair.
  * PSUM (8 banks) split so all phases coexist: scores 5x[128,512]
    (5-slot pool decouples slot recycling from the exp drain order),
    res+s accumulators 2, projection scratch 1.
  * Single global software pipeline over the head-pair stream (across
    batch boundaries): pair g's scores+exp overlap pair g-1's AV
    matmuls; qkv/v projections of batch i+1 and the out-projection of
    batch i-1 are queued as filler chunks, one per step, placed BEFORE
    the score matmuls that wait on exp PSUM-slot releases so the
    in-order PE queue never stalls on ready work.
  * exp(x/8) on ScalarE with fused scale, no max-subtraction (scores are
    ~N(0,1)); softmax denominator via ones-stationary matmuls sharing
    the AV accumulation; 1/s via DVE reciprocal_approx_fast.
  * bf16 everywhere except the q/k tiles and score matmuls (f32r): x,
    w_qkv, w_v, w_out, AV operands and rt. All bf16 stationaries are
    64-col (split col-tiled halves for the projections) -- LDWEIGHTS
    fully hides under the streams and the 128-col fast-weight-load
    corruption path is never engaged. Final rel err ~3e-4.
"""

from collections import deque
from contextlib import ExitStack

import ml_dtypes
import numpy as np

import concourse.bacc as bacc
import concourse.bass as bass
import concourse.tile as tile
from concourse import mybir
from concourse.bass_utils import run_bass_kernel_spmd  # noqa: F401 (fallback path)

N_CORES = 8
B, HH, WW, C = 16, 32, 32, 512
N = HH * WW            # 1024 tokens
NH, DH = 8, 64
SCALE = DH ** -0.5     # 0.125
BL = B // N_CORES      # 2 batch elements per core
P = 128
F32 = mybir.dt.float32
F32R = mybir.dt.float32r
MM_DT = F32R
AV_DT = mybir.dt.bfloat16


def build_program(repeat=1) -> bass.Bass:
    inner, hw_loop = repeat if isinstance(repeat, tuple) else (repeat, None)
    nc = bacc.Bacc("TRN2", target_bir_lowering=False, debug=False)

    xT_d = nc.dram_tensor("xT", [BL, C, N], AV_DT, kind="ExternalInput")
    rs_d = nc.dram_tensor("resid", [BL, N, C], F32, kind="ExternalInput")
    wqk_d = nc.dram_tensor("wqk", [C, 1024], AV_DT, kind="ExternalInput")
    bqk_d = nc.dram_tensor("bqk", [1024], F32, kind="ExternalInput")
    wv_d = nc.dram_tensor("wv", [C, 512], AV_DT, kind="ExternalInput")
    wout_d = nc.dram_tensor("wout", [C, 512], AV_DT, kind="ExternalInput")
    out_d = nc.dram_tensor("out", [BL, N, C], F32, kind="ExternalOutput")

    with tile.TileContext(nc) as tc, ExitStack() as ctx:
        consts = ctx.enter_context(tc.tile_pool(name="consts", bufs=1))
        wpool = ctx.enter_context(tc.tile_pool(name="w", bufs=1))
        xt_pool = ctx.enter_context(tc.tile_pool(name="xt", bufs=8))
        qk_pool = ctx.enter_context(tc.tile_pool(name="qk", bufs=13))
        v_pool = ctx.enter_context(tc.tile_pool(name="v", bufs=16))
        ex_pool = ctx.enter_context(tc.tile_pool(name="ex", bufs=54))
        rt_pool = ctx.enter_context(tc.tile_pool(name="rt", bufs=22))
        misc = ctx.enter_context(tc.tile_pool(name="misc", bufs=2))
        # PSUM: 8 banks. scores 5x[128,512] (separate tiles per head and
        # x-half -- merging heads into one tile serializes the score
        # matmuls); res+s accumulators = 2; projections = 1.
        psb = ctx.enter_context(tc.tile_pool(name="psb", bufs=5, space="PSUM"))
        pst = ctx.enter_context(tc.tile_pool(name="pst", bufs=1, space="PSUM"))
        psp = ctx.enter_context(tc.tile_pool(name="psp", bufs=1, space="PSUM"))

        ones = consts.tile([P, 64], AV_DT, tag="ones")
        nc.vector.memset(ones[:], 1.0)
        bqk_sb = consts.tile([P, 8], F32, tag="bqk")
        nc.sync.dma_start(out=bqk_sb[:], in_=bqk_d.ap().rearrange("(t p) -> p t", p=P))

        wqk_sb, wv_sb, wout_sb = [], [], []
        for kc in range(4):
            t = wpool.tile([P, 1024], AV_DT, tag=f"wqk{kc}")
            nc.sync.dma_start(out=t[:], in_=wqk_d.ap()[kc * P:(kc + 1) * P, :])
            wqk_sb.append(t)
        for kc in range(4):
            t = wpool.tile([P, 512], AV_DT, tag=f"wv{kc}")
            nc.sync.dma_start(out=t[:], in_=wv_d.ap()[kc * P:(kc + 1) * P, :])
            wv_sb.append(t)
            t = wpool.tile([P, 512], AV_DT, tag=f"wout{kc}")
            nc.sync.dma_start(out=t[:], in_=wout_d.ap()[kc * P:(kc + 1) * P, :])
            wout_sb.append(t)

        # per-live-batch state, keyed by linear batch index i
        xt = {}      # i -> [4 tiles]
        qk = {}      # (i, jt) -> tile
        vo = {}      # (i, yt) -> tile
        ex = {}      # (i, p, yt, e, xs) -> tile
        rt = {}      # (i, p, xs) -> tile
        acc = {}     # (i, p, xs) -> (res_ps, s_ps)

        def emit_xt_dma(i):
            b = i % BL
            xt[i] = []
            for kc in range(4):
                t = xt_pool.tile([P, N], AV_DT)
                nc.sync.dma_start(out=t[:], in_=xT_d.ap()[b, kc * P:(kc + 1) * P, :])
                xt[i].append(t)

        def emit_qk_chunk(i, jt, xs):
            # bf16 stationaries split into 64-col halves: avoids the
            # 128-col fast-weight-load path, halves+hides LDWEIGHTS, and
            # the two halves run col-tiled concurrently.
            ps = psp.tile([P, 512], F32, tag="p")
            for kc in range(4):
                for e2 in range(2):
                    nc.tensor.matmul(
                        ps[e2 * 64:(e2 + 1) * 64, :],
                        wqk_sb[kc][:, jt * P + e2 * 64:jt * P + (e2 + 1) * 64],
                        xt[i][kc][:, xs * 512:(xs + 1) * 512],
                        start=(kc == 0), stop=(kc == 3), skip_group_check=True,
                    )
            if xs == 0:
                qk[(i, jt)] = qk_pool.tile([P, N], MM_DT, tag="qk",
                                           name=f"qk_{i}_{jt}")
            nc.vector.tensor_scalar(
                out=qk[(i, jt)][:, xs * 512:(xs + 1) * 512], in0=ps[:],
                scalar1=bqk_sb[:, jt:jt + 1], scalar2=None,
                op0=mybir.AluOpType.add,
            )

        def emit_v_chunk(i, yt):
            ps = psp.tile([P, 512], F32, tag="p")
            for kc in range(4):
                for e2 in range(2):
                    nc.tensor.matmul(
                        ps[e2 * 64:(e2 + 1) * 64, :],
                        xt[i][kc][:, yt * P + e2 * 64:yt * P + (e2 + 1) * 64],
                        wv_sb[kc][:],
                        start=(kc == 0), stop=(kc == 3), skip_group_check=True,
                    )
            t = v_pool.tile([P, 512], AV_DT)
            nc.vector.tensor_copy(t[:], ps[:])
            vo[(i, yt)] = t

        # Scores for one (pair, yt) step: 4 PSUM tiles [128,512] keyed
        # (e, xs), allocated in the order (e0,x0),(e1,x0),(e0,x1),(e1,x1)
        # to match the exp drain order; the matmul pairs (e0,x) + (e1,x)
        # are adjacent so the PE row-tiles them concurrently (K=64 heads
        # at row offsets 0 / 64).
        def emit_scores(i, p, yt):
            kk = qk[(i, p)]
            qq = qk[(i, 4 + p)]
            sc = {}
            for xs in range(2):
                for e in range(2):
                    sc[(e, xs)] = psb.tile([P, 512], F32, tag="big",
                                           name=f"sc_{e}_{xs}")
            for xs in range(2):
                for e in range(2):
                    nc.tensor.matmul(
                        sc[(e, xs)][:],
                        kk[e * 64:(e + 1) * 64, yt * P:(yt + 1) * P],
                        qq[e * 64:(e + 1) * 64, xs * 512:(xs + 1) * 512],
                        start=True, stop=True,
                    )
            for xs in range(2):
                for e in range(2):
                    t = ex_pool.tile([P, 512], AV_DT, tag="ex",
                                     name=f"ex_{e}_{xs}")
                    nc.scalar.activation(
                        out=t[:], in_=sc[(e, xs)][:],
                        func=mybir.ActivationFunctionType.Exp, scale=SCALE,
                    )
                    ex[(i, p, yt, e, xs)] = t

        # AV chunks for a (pair, xs) group are emitted in y-order
        # 1,0,3,2,5,4,7,6 (so the group-opening chunk never lands right
        # behind the divide that frees its PSUM slot): accumulation-group
        # flags go on the first/last EMITTED chunk.
        def emit_av_chunk(i, p, xs, yt):
            if yt == 1:
                res_ps = pst.tile([P, 512], F32, tag="res")
                s_ps = pst.tile([P, 512], F32, tag="s")
                acc[(i, p, xs)] = (res_ps, s_ps)
            res_ps, s_ps = acc[(i, p, xs)]
            first, last = yt == 1, yt == 6
            exs = [ex[(i, p, yt, e, xs)][:] for e in range(2)]
            for e in range(2):
                h = 2 * p + e
                nc.tensor.matmul(
                    res_ps[e * 64:(e + 1) * 64, :],
                    vo[(i, yt)][:, h * 64:(h + 1) * 64],
                    exs[e],
                    start=first, stop=last, skip_group_check=True,
                )
            for e in range(2):
                nc.tensor.matmul(
                    s_ps[e * 64:(e + 1) * 64, :],
                    ones[:],
                    exs[e],
                    start=first, stop=last, skip_group_check=True,
                )

        def emit_divide(i, p, xs):
            res_ps, s_ps = acc.pop((i, p, xs))
            rec = misc.tile([P, 512], F32, tag="prc")
            nc.vector.reciprocal_approx_fast(out=rec[:], in_=s_ps[:])
            t = rt_pool.tile([P, 512], AV_DT)
            nc.vector.tensor_tensor(
                out=t[:], in0=res_ps[:], in1=rec[:],
                op=mybir.AluOpType.mult,
            )
            rt[(i, p, xs)] = t
            if xs == 1:
                for yy in range(8):
                    for e in range(2):
                        for xx in range(2):
                            del ex[(i, p, yy, e, xx)]
                if p == 3:
                    for yy in range(8):
                        del vo[(i, yy)]

        def emit_out_chunk(i, nt):
            b = i % BL
            xs, sub = divmod(nt, 4)
            ps = psp.tile([P, 512], F32, tag="p")
            # rt stationaries split into 64-col halves: avoids the 128-col
            # bf16 fast-weight-load corruption and the two halves run
            # col-tiled concurrently (out partitions 0-63 / 64-127).
            for p in range(4):
                for e in range(2):
                    nc.tensor.matmul(
                        ps[e * 64:(e + 1) * 64, :],
                        rt[(i, p, xs)][:, sub * P + e * 64:sub * P + (e + 1) * 64],
                        wout_sb[p][:],
                        start=(p == 0), stop=(p == 3), skip_group_check=True,
                    )
            rs = misc.tile([P, 512], F32, tag="rs")
            nc.sync.dma_start(out=rs[:], in_=rs_d.ap()[b, nt * P:(nt + 1) * P, :])
            ob = misc.tile([P, 512], F32, tag="ob")
            nc.vector.tensor_tensor(
                out=ob[:], in0=ps[:], in1=rs[:], op=mybir.AluOpType.add,
            )
            nc.sync.dma_start(out=out_d.ap()[b, nt * P:(nt + 1) * P, :], in_=ob[:])
            if nt == 7:
                for p in range(4):
                    for xs2 in range(2):
                        del rt[(i, p, xs2)]

        def _batches():
            nb = inner * BL
            filler = deque()

            def pump(k):
                for _ in range(k):
                    if not filler:
                        return
                    filler.popleft()()

            # prologue: first batch's input + projections emitted directly
            emit_xt_dma(0)
            for jt in range(8):
                for xs in range(2):
                    emit_qk_chunk(0, jt, xs)
            for yt in range(8):
                emit_v_chunk(0, yt)
            if nb > 1:
                emit_xt_dma(1)

            def queue_batch_work(i):
                # Fillers consumed during batch i's 32 steps (1 per step):
                # projections for batch i+1 interleaved so qk tile slot
                # releases (one pair's kk+qq free every 8 steps) line up,
                # then the out-projection of batch i-1 (whose rt tiles
                # finish during this batch's pair-0 window).
                if i + 1 < nb:
                    if i + 2 < nb:
                        emit_xt_dma(i + 2)
                    def qkc(jt):
                        return [(lambda jt=jt, xs=xs: emit_qk_chunk(i + 1, jt, xs))
                                for xs in range(2)]
                    def vc(y0, y1):
                        return [(lambda yt=yt: emit_v_chunk(i + 1, yt))
                                for yt in range(y0, y1)]
                    for c in (qkc(0) + qkc(1) + qkc(2) + qkc(3) + vc(0, 4)
                              + qkc(4) + qkc(5) + vc(4, 8) + qkc(6) + qkc(7)):
                        filler.append(c)

            # Global pair stream: pair g's scores+exp overlap pair (g-1)'s
            # AV matmuls, across batch boundaries. Per-step PE order
            # [filler, AV, scores] keeps ready work ahead of the score
            # matmuls that wait on exp slot releases.
            G = 4 * nb
            for g in range(G):
                i, p = divmod(g, 4)
                if p == 0:
                    queue_batch_work(i)
                if p == 1 and i >= 1:
                    # batch i-1's rt tiles are complete once pair 0's AV
                    # divides have been emitted (during pair 0's steps)
                    filler.extend(
                        (lambda nt=nt: emit_out_chunk(i - 1, nt))
                        for nt in range(8))
                for yt in range(8):
                    xs, k = divmod(yt, 4)
                    if g >= 1 and k == 3:
                        # group-final step: AV + divide ahead of the filler
                        # so the reciprocal leads the DVE queue -- the next
                        # AV group's opening matmul waits on the PSUM slot
                        # this divide releases.
                        qi, qp = divmod(g - 1, 4)
                        emit_av_chunk(qi, qp, xs, 2 * k + 1)
                        emit_av_chunk(qi, qp, xs, 2 * k)
                        emit_divide(qi, qp, xs)
                        pump(1)
                    else:
                        pump(1)
                        if g >= 1:
                            qi, qp = divmod(g - 1, 4)
                            emit_av_chunk(qi, qp, xs, 2 * k + 1)
                            emit_av_chunk(qi, qp, xs, 2 * k)
                    emit_scores(i, p, yt)
            # epilogue: drain the last pair's AV + out-projection
            qi, qp = nb - 1, 3
            for xs in range(2):
                for k in range(4):
                    emit_av_chunk(qi, qp, xs, 2 * k + 1)
                    emit_av_chunk(qi, qp, xs, 2 * k)
                emit_divide(qi, qp, xs)
            for nt in range(8):
                emit_out_chunk(nb - 1, nt)
            pump(len(filler))

        if hw_loop:
            with tc.For_i(0, hw_loop):
                _batches()
        else:
            _batches()
    nc.compile()
    return nc


def host_prep(ft, w_qkv, b_qkv, w_out, b_out):
    ft = np.asarray(ft, dtype=np.float32)
    w_qkv = np.asarray(w_qkv, dtype=np.float32)
    b_qkv = np.asarray(b_qkv, dtype=np.float32)
    w_out = np.asarray(w_out, dtype=np.float32)
    b_out = np.asarray(b_out, dtype=np.float32)

    x = ft.reshape(B, N, C)
    xT = np.ascontiguousarray(x.transpose(0, 2, 1))

    w_qk_re = np.empty((C, 1024), np.float32)
    b_qk_re = np.empty((1024,), np.float32)
    w_v_re = np.empty((C, 512), np.float32)
    for p in range(4):
        hA, hB = 2 * p, 2 * p + 1
        w_qk_re[:, p * 128:p * 128 + 64] = w_qkv[:, hA * 192 + 64:hA * 192 + 128]
        w_qk_re[:, p * 128 + 64:p * 128 + 128] = w_qkv[:, hB * 192 + 64:hB * 192 + 128]
        b_qk_re[p * 128:p * 128 + 64] = b_qkv[hA * 192 + 64:hA * 192 + 128]
        b_qk_re[p * 128 + 64:p * 128 + 128] = b_qkv[hB * 192 + 64:hB * 192 + 128]
        w_qk_re[:, 512 + p * 128:512 + p * 128 + 64] = w_qkv[:, hA * 192:hA * 192 + 64]
        w_qk_re[:, 512 + p * 128 + 64:512 + p * 128 + 128] = w_qkv[:, hB * 192:hB * 192 + 64]
        b_qk_re[512 + p * 128:512 + p * 128 + 64] = b_qkv[hA * 192:hA * 192 + 64]
        b_qk_re[512 + p * 128 + 64:512 + p * 128 + 128] = b_qkv[hB * 192:hB * 192 + 64]
        w_v_re[:, p * 128:p * 128 + 64] = w_qkv[:, hA * 192 + 128:hA * 192 + 192]
        w_v_re[:, p * 128 + 64:p * 128 + 128] = w_qkv[:, hB * 192 + 128:hB * 192 + 192]

    b_v_nat = np.empty((512,), np.float32)
    for h in range(NH):
        b_v_nat[h * 64:(h + 1) * 64] = b_qkv[h * 192 + 128:h * 192 + 192]
    resid = x + b_out[None, None, :] + (b_v_nat @ w_out)[None, None, :]
    resid = np.ascontiguousarray(resid, dtype=np.float32)
    return xT, resid, w_qk_re, b_qk_re, w_v_re, np.ascontiguousarray(w_out)


_NC_CACHE = {}


def get_program(repeat: int = 1) -> bass.Bass:
    if repeat not in _NC_CACHE:
        _NC_CACHE[repeat] = build_program(repeat)
    return _NC_CACHE[repeat]


def make_in_maps(ft, w_qkv, b_qkv, w_out, b_out):
    xT, resid, w_qk_re, b_qk_re, w_v_re, w_out_c = host_prep(
        ft, w_qkv, b_qkv, w_out, b_out)
    in_maps = []
    for core in range(N_CORES):
        sl = slice(core * BL, (core + 1) * BL)
        in_maps.append({
            "xT": np.ascontiguousarray(xT[sl].astype(ml_dtypes.bfloat16)),
            "resid": np.ascontiguousarray(resid[sl]),
            "wqk": np.ascontiguousarray(w_qk_re.astype(ml_dtypes.bfloat16)),
            "bqk": b_qk_re,
            "wv": np.ascontiguousarray(w_v_re.astype(ml_dtypes.bfloat16)),
            "wout": np.ascontiguousarray(w_out_c.astype(ml_dtypes.bfloat16)),
        })
    return in_maps


_RUNNER_CACHE = {}


def make_runner(repeat: int = 1):
    """Build (once) a persistent jitted executor for the bass program.

    Returns run(in_maps) -> list[dict[name, np.ndarray]] per core. Keeping
    the jitted callable alive means repeat calls skip lowering/compile and
    are pure dispatch+execute.
    """
    if repeat in _RUNNER_CACHE:
        return _RUNNER_CACHE[repeat]

    import jax
    from jax.experimental.shard_map import shard_map
    from jax.sharding import Mesh, PartitionSpec
    from concourse import mybir as _mb
    from concourse import bass2jax

    bass2jax.install_neuronx_cc_hook()
    nc = get_program(repeat)

    partition_name = nc.partition_id_tensor.name if nc.partition_id_tensor else None
    in_names, out_names, out_avals, zero_shapes = [], [], [], []
    for alloc in nc.m.functions[0].allocations:
        if not isinstance(alloc, _mb.MemoryLocationSet):
            continue
        name = alloc.memorylocations[0].name
        if alloc.kind == "ExternalInput":
            if name != partition_name:
                in_names.append(name)
        elif alloc.kind == "ExternalOutput":
            np_dt = _mb.dt.np(alloc.dtype)
            out_names.append(name)
            out_avals.append(jax.core.ShapedArray(tuple(alloc.tensor_shape), np_dt))
            zero_shapes.append((tuple(alloc.tensor_shape), np_dt))
    n_params = len(in_names)
    all_in_names = list(in_names) + list(out_names)
    if partition_name is not None:
        all_in_names.append(partition_name)

    def _body(*args):
        operands = list(args)
        if partition_name is not None:
            operands.append(bass2jax.partition_id_tensor())
        outs = bass2jax._bass_exec_p.bind(
            *operands,
            out_avals=tuple(out_avals),
            in_names=tuple(all_in_names),
            out_names=tuple(out_names),
            lowering_input_output_aliases=(),
            sim_require_finite=True,
            sim_require_nnan=True,
            nc=nc,
        )
        return tuple(outs)

    devices = jax.devices()[:N_CORES]
    mesh = Mesh(np.asarray(devices), ("core",))
    n_outs = len(out_names)
    sharded = jax.jit(
        shard_map(_body, mesh=mesh,
                  in_specs=(PartitionSpec("core"),) * (n_params + n_outs),
                  out_specs=(PartitionSpec("core"),) * n_outs,
                  check_rep=False),
        keep_unused=True,
    )

    def run(in_maps):
        concat_in = [
            np.concatenate([np.asarray(m[name]) for m in in_maps], axis=0)
            for name in in_names
        ]
        zeros = [np.zeros((N_CORES * s[0], *s[1:]), dt) for s, dt in zero_shapes]
        out_arrs = sharded(*concat_in, *zeros)
        return [
            {name: np.asarray(out_arrs[i]).reshape(N_CORES, *out_avals[i].shape)[c]
             for i, name in enumerate(out_names)}
            for c in range(N_CORES)
        ]

    def make_chained(k):
        def _chain(*args):
            ins = list(args[:n_params])
            bufs = list(args[n_params:])
            for _ in range(k):
                bufs = list(_body(*ins, *bufs))
            return tuple(bufs)
        return jax.jit(
            shard_map(_chain, mesh=mesh,
                      in_specs=(PartitionSpec("core"),) * (n_params + n_outs),
                      out_specs=(PartitionSpec("core"),) * n_outs,
                      check_rep=False),
            keep_unused=True,
        )

    run.sharded = sharded
    run.in_names = in_names
    run.zero_shapes = zero_shapes
    run.make_chained = make_chained
    run.mesh = mesh
    _RUNNER_CACHE[repeat] = run
    return run


def kernel(ft, w_qkv, b_qkv, w_out, b_out):
    run = make_runner()
    in_maps = make_in_maps(ft, w_qkv, b_qkv, w_out, b_out)
    results = run(in_maps)
    out = np.concatenate([r["out"] for r in results], axis=0)
    return out.reshape(B, HH, WW, C).astype(np.float32)


# revision 49
# speedup vs baseline: 1.0943x; 1.0168x over previous
"""Trainium2 Bass kernel for nn_Attention_85564338471023.

Multi-head self-attention (B=16, N=1024 tokens, C=512, 8 heads x d=64) with
qkv projection, softmax attention, output projection and residual.

Sharding: pure data-parallel over batch -- 2 batch elements per NeuronCore,
no collectives. Host pre-transposes x (channels-on-partitions) and reorders
w_qkv columns so heads come in pairs that share 128-partition tiles.

Pipeline (204us baseline -> ~140us):
  * Head-pair concurrency on the PE: the two heads' score matmuls (K=64
    stationaries at row offsets 0/64) and AV/denominator matmuls (M=64
    outputs at partition offsets 0/64) are emitted adjacently so the PE
    row-/col-tiles them into disjoint quadrants and streams them
    concurrently. Score tiles must stay SEPARATE PSUM tiles per head --
    merging both heads into one [128,1024] tile serializes the pair.
  * PSUM (8 banks) split so all phases coexist: scores 5x[128,512]
    (5-slot pool decouples slot recycling from the exp drain order),
    res+s accumulators 2, projection scratch 1.
  * Single global software pipeline over the head-pair stream (across
    batch boundaries): pair g's scores+exp overlap pair g-1's AV
    matmuls; qkv/v projections of batch i+1 and the out-projection of
    batch i-1 are queued as filler chunks, one per step, placed BEFORE
    the score matmuls that wait on exp PSUM-slot releases so the
    in-order PE queue never stalls on ready work.
  * exp(x/8) on ScalarE with fused scale, no max-subtraction (scores are
    ~N(0,1)); softmax denominator via ones-stationary matmuls sharing
    the AV accumulation; 1/s via DVE reciprocal_approx_fast.
  * bf16 everywhere except the q/k tiles and score matmuls (f32r): x,
    w_qkv, w_v, w_out, AV operands and rt. All bf16 stationaries are
    64-col (split col-tiled halves for the projections) -- LDWEIGHTS
    fully hides under the streams and the 128-col fast-weight-load
    corruption path is never engaged. Final rel err ~3e-4.
"""

from collections import deque
from contextlib import ExitStack

import ml_dtypes
import numpy as np

import concourse.bacc as bacc
import concourse.bass as bass
import concourse.tile as tile
from concourse import mybir
from concourse.bass_utils import run_bass_kernel_spmd  # noqa: F401 (fallback path)

N_CORES = 8
B, HH, WW, C = 16, 32, 32, 512
N = HH * WW            # 1024 tokens
NH, DH = 8, 64
SCALE = DH ** -0.5     # 0.125
BL = B // N_CORES      # 2 batch elements per core
P = 128
F32 = mybir.dt.float32
F32R = mybir.dt.float32r
MM_DT = F32R
AV_DT = mybir.dt.bfloat16


def build_program(repeat=1) -> bass.Bass:
    inner, hw_loop = repeat if isinstance(repeat, tuple) else (repeat, None)
    nc = bacc.Bacc("TRN2", target_bir_lowering=False, debug=False)

    xT_d = nc.dram_tensor("xT", [BL, C, N], AV_DT, kind="ExternalInput")
    rs_d = nc.dram_tensor("resid", [BL, N, C], F32, kind="ExternalInput")
    wqk_d = nc.dram_tensor("wqk", [C, 1024], AV_DT, kind="ExternalInput")
    bqk_d = nc.dram_tensor("bqk", [1024], F32, kind="ExternalInput")
    wv_d = nc.dram_tensor("wv", [C, 512], AV_DT, kind="ExternalInput")
    wout_d = nc.dram_tensor("wout", [C, 512], AV_DT, kind="ExternalInput")
    out_d = nc.dram_tensor("out", [BL, N, C], F32, kind="ExternalOutput")

    with tile.TileContext(nc) as tc, ExitStack() as ctx:
        consts = ctx.enter_context(tc.tile_pool(name="consts", bufs=1))
        wpool = ctx.enter_context(tc.tile_pool(name="w", bufs=1))
        xt_pool = ctx.enter_context(tc.tile_pool(name="xt", bufs=8))
        qk_pool = ctx.enter_context(tc.tile_pool(name="qk", bufs=13))
        v_pool = ctx.enter_context(tc.tile_pool(name="v", bufs=16))
        ex_pool = ctx.enter_context(tc.tile_pool(name="ex", bufs=54))
        rt_pool = ctx.enter_context(tc.tile_pool(name="rt", bufs=22))
        misc = ctx.enter_context(tc.tile_pool(name="misc", bufs=2))
        # PSUM: 8 banks. scores 5x[128,512] (separate tiles per head and
        # x-half -- merging heads into one tile serializes the score
        # matmuls); res+s accumulators = 2; projections = 1.
        psb = ctx.enter_context(tc.tile_pool(name="psb", bufs=5, space="PSUM"))
        pst = ctx.enter_context(tc.tile_pool(name="pst", bufs=1, space="PSUM"))
        psp = ctx.enter_context(tc.tile_pool(name="psp", bufs=1, space="PSUM"))

        ones = consts.tile([P, 64], AV_DT, tag="ones")
        nc.vector.memset(ones[:], 1.0)
        bqk_sb = consts.tile([P, 8], F32, tag="bqk")
        nc.sync.dma_start(out=bqk_sb[:], in_=bqk_d.ap().rearrange("(t p) -> p t", p=P))

        wqk_sb, wv_sb, wout_sb = [], [], []
        for kc in range(4):
            t = wpool.tile([P, 1024], AV_DT, tag=f"wqk{kc}")
            nc.sync.dma_start(out=t[:], in_=wqk_d.ap()[kc * P:(kc + 1) * P, :])
            wqk_sb.append(t)
        for kc in range(4):
            t = wpool.tile([P, 512], AV_DT, tag=f"wv{kc}")
            nc.sync.dma_start(out=t[:], in_=wv_d.ap()[kc * P:(kc + 1) * P, :])
            wv_sb.append(t)
            t = wpool.tile([P, 512], AV_DT, tag=f"wout{kc}")
            nc.sync.dma_start(out=t[:], in_=wout_d.ap()[kc * P:(kc + 1) * P, :])
            wout_sb.append(t)

        # per-live-batch state, keyed by linear batch index i
        xt = {}      # i -> [4 tiles]
        qk = {}      # (i, jt) -> tile
        vo = {}      # (i, yt) -> tile
        ex = {}      # (i, p, yt, e, xs) -> tile
        rt = {}      # (i, p, xs) -> tile
        acc = {}     # (i, p, xs) -> (res_ps, s_ps)

        def emit_xt_dma(i):
            b = i % BL
            xt[i] = []
            for kc in range(4):
                t = xt_pool.tile([P, N], AV_DT)
                nc.sync.dma_start(out=t[:], in_=xT_d.ap()[b, kc * P:(kc + 1) * P, :])
                xt[i].append(t)

        def emit_qk_chunk(i, jt, xs):
            # bf16 stationaries split into 64-col halves: avoids the
            # 128-col fast-weight-load path, halves+hides LDWEIGHTS, and
            # the two halves run col-tiled concurrently.
            ps = psp.tile([P, 512], F32, tag="p")
            for kc in range(4):
                for e2 in range(2):
                    nc.tensor.matmul(
                        ps[e2 * 64:(e2 + 1) * 64, :],
                        wqk_sb[kc][:, jt * P + e2 * 64:jt * P + (e2 + 1) * 64],
                        xt[i][kc][:, xs * 512:(xs + 1) * 512],
                        start=(kc == 0), stop=(kc == 3), skip_group_check=True,
                    )
            if xs == 0:
                qk[(i, jt)] = qk_pool.tile([P, N], MM_DT, tag="qk",
                                           name=f"qk_{i}_{jt}")
            nc.vector.tensor_scalar(
                out=qk[(i, jt)][:, xs * 512:(xs + 1) * 512], in0=ps[:],
                scalar1=bqk_sb[:, jt:jt + 1], scalar2=None,
                op0=mybir.AluOpType.add,
            )

        def emit_v_chunk(i, yt):
            ps = psp.tile([P, 512], F32, tag="p")
            for kc in range(4):
                for e2 in range(2):
                    nc.tensor.matmul(
                        ps[e2 * 64:(e2 + 1) * 64, :],
                        xt[i][kc][:, yt * P + e2 * 64:yt * P + (e2 + 1) * 64],
                        wv_sb[kc][:],
                        start=(kc == 0), stop=(kc == 3), skip_group_check=True,
                    )
            t = v_pool.tile([P, 512], AV_DT)
            nc.vector.tensor_copy(t[:], ps[:])
            vo[(i, yt)] = t

        # Scores for one (pair, yt) step: 4 PSUM tiles [128,512] keyed
        # (e, xs), allocated in the order (e0,x0),(e1,x0),(e0,x1),(e1,x1)
        # to match the exp drain order; the matmul pairs (e0,x) + (e1,x)
        # are adjacent so the PE row-tiles them concurrently (K=64 heads
        # at row offsets 0 / 64).
        def emit_scores(i, p, yt):
            kk = qk[(i, p)]
            qq = qk[(i, 4 + p)]
            sc = {}
            for xs in range(2):
                for e in range(2):
                    sc[(e, xs)] = psb.tile([P, 512], F32, tag="big",
                                           name=f"sc_{e}_{xs}")
            for xs in range(2):
                for e in range(2):
                    nc.tensor.matmul(
                        sc[(e, xs)][:],
                        kk[e * 64:(e + 1) * 64, yt * P:(yt + 1) * P],
                        qq[e * 64:(e + 1) * 64, xs * 512:(xs + 1) * 512],
                        start=True, stop=True,
                    )
            for xs in range(2):
                for e in range(2):
                    t = ex_pool.tile([P, 512], AV_DT, tag="ex",
                                     name=f"ex_{e}_{xs}")
                    nc.scalar.activation(
                        out=t[:], in_=sc[(e, xs)][:],
                        func=mybir.ActivationFunctionType.Exp, scale=SCALE,
                    )
                    ex[(i, p, yt, e, xs)] = t

        # AV chunks for a (pair, xs) group are emitted in y-order
        # 1,0,3,2,5,4,7,6 (so the group-opening chunk never lands right
        # behind the divide that frees its PSUM slot): accumulation-group
        # flags go on the first/last EMITTED chunk.
        def emit_av_chunk(i, p, xs, yt):
            if yt == 1:
                res_ps = pst.tile([P, 512], F32, tag="res")
                s_ps = pst.tile([P, 512], F32, tag="s")
                acc[(i, p, xs)] = (res_ps, s_ps)
            res_ps, s_ps = acc[(i, p, xs)]
            first, last = yt == 1, yt == 6
            exs = [ex[(i, p, yt, e, xs)][:] for e in range(2)]

            def res_mms():
                for e in range(2):
                    h = 2 * p + e
                    nc.tensor.matmul(
                        res_ps[e * 64:(e + 1) * 64, :],
                        vo[(i, yt)][:, h * 64:(h + 1) * 64],
                        exs[e],
                        start=first, stop=last, skip_group_check=True,
                    )

            def s_mms():
                for e in range(2):
                    nc.tensor.matmul(
                        s_ps[e * 64:(e + 1) * 64, :],
                        ones[:],
                        exs[e],
                        start=first, stop=last, skip_group_check=True,
                    )

            # group-final chunk: s stops first so the divide's reciprocal
            # overlaps the trailing res-pair stream
            if last:
                s_mms()
                res_mms()
            else:
                res_mms()
                s_mms()

        def emit_divide(i, p, xs):
            res_ps, s_ps = acc.pop((i, p, xs))
            rec = misc.tile([P, 512], F32, tag="prc")
            nc.vector.reciprocal_approx_fast(out=rec[:], in_=s_ps[:])
            t = rt_pool.tile([P, 512], AV_DT)
            nc.vector.tensor_tensor(
                out=t[:], in0=res_ps[:], in1=rec[:],
                op=mybir.AluOpType.mult,
            )
            rt[(i, p, xs)] = t
            if xs == 1:
                for yy in range(8):
                    for e in range(2):
                        for xx in range(2):
                            del ex[(i, p, yy, e, xx)]
                if p == 3:
                    for yy in range(8):
                        del vo[(i, yy)]

        def emit_out_chunk(i, nt):
            b = i % BL
            xs, sub = divmod(nt, 4)
            ps = psp.tile([P, 512], F32, tag="p")
            # rt stationaries split into 64-col halves: avoids the 128-col
            # bf16 fast-weight-load corruption and the two halves run
            # col-tiled concurrently (out partitions 0-63 / 64-127).
            for p in range(4):
                for e in range(2):
                    nc.tensor.matmul(
                        ps[e * 64:(e + 1) * 64, :],
                        rt[(i, p, xs)][:, sub * P + e * 64:sub * P + (e + 1) * 64],
                        wout_sb[p][:],
                        start=(p == 0), stop=(p == 3), skip_group_check=True,
                    )
            rs = misc.tile([P, 512], F32, tag="rs")
            nc.sync.dma_start(out=rs[:], in_=rs_d.ap()[b, nt * P:(nt + 1) * P, :])
            ob = misc.tile([P, 512], F32, tag="ob")
            nc.vector.tensor_tensor(
                out=ob[:], in0=ps[:], in1=rs[:], op=mybir.AluOpType.add,
            )
            nc.sync.dma_start(out=out_d.ap()[b, nt * P:(nt + 1) * P, :], in_=ob[:])
            if nt == 7:
                for p in range(4):
                    for xs2 in range(2):
                        del rt[(i, p, xs2)]

        def _batches():
            nb = inner * BL
            filler = deque()

            def pump(k):
                for _ in range(k):
                    if not filler:
                        return
                    filler.popleft()()

            # prologue: first batch's input + projections emitted directly
            emit_xt_dma(0)
            for jt in range(8):
                for xs in range(2):
                    emit_qk_chunk(0, jt, xs)
            for yt in range(8):
                emit_v_chunk(0, yt)
            if nb > 1:
                emit_xt_dma(1)

            def queue_batch_work(i):
                # Fillers consumed during batch i's 32 steps (1 per step):
                # projections for batch i+1 interleaved so qk tile slot
                # releases (one pair's kk+qq free every 8 steps) line up,
                # then the out-projection of batch i-1 (whose rt tiles
                # finish during this batch's pair-0 window).
                if i + 1 < nb:
                    if i + 2 < nb:
                        emit_xt_dma(i + 2)
                    def qkc(jt):
                        return [(lambda jt=jt, xs=xs: emit_qk_chunk(i + 1, jt, xs))
                                for xs in range(2)]
                    def vc(y0, y1):
                        return [(lambda yt=yt: emit_v_chunk(i + 1, yt))
                                for yt in range(y0, y1)]
                    for c in (qkc(0) + qkc(1) + qkc(2) + qkc(3) + vc(0, 4)
                              + qkc(4) + qkc(5) + vc(4, 8) + qkc(6) + qkc(7)):
                        filler.append(c)

            # Global pair stream: pair g's scores+exp overlap pair (g-1)'s
            # AV matmuls, across batch boundaries. Per-step PE order
            # [filler, AV, scores] keeps ready work ahead of the score
            # matmuls that wait on exp slot releases.
            G = 4 * nb
            for g in range(G):
                i, p = divmod(g, 4)
                if p == 0:
                    queue_batch_work(i)
                if p == 1 and i >= 1:
                    # batch i-1's rt tiles are complete once pair 0's AV
                    # divides have been emitted (during pair 0's steps)
                    filler.extend(
                        (lambda nt=nt: emit_out_chunk(i - 1, nt))
                        for nt in range(8))
                for yt in range(8):
                    xs, k = divmod(yt, 4)
                    if g >= 1 and k == 3:
                        # group-final step: AV + divide ahead of the filler
                        # so the reciprocal leads the DVE queue -- the next
                        # AV group's opening matmul waits on the PSUM slot
                        # this divide releases.
                        qi, qp = divmod(g - 1, 4)
                        emit_av_chunk(qi, qp, xs, 2 * k + 1)
                        emit_av_chunk(qi, qp, xs, 2 * k)
                        emit_divide(qi, qp, xs)
                        pump(1)
                    else:
                        pump(1)
                        if g >= 1:
                            qi, qp = divmod(g - 1, 4)
                            emit_av_chunk(qi, qp, xs, 2 * k + 1)
                            emit_av_chunk(qi, qp, xs, 2 * k)
                    emit_scores(i, p, yt)
            # epilogue: drain the last pair's AV + out-projection
            qi, qp = nb - 1, 3
            for xs in range(2):
                for k in range(4):
                    emit_av_chunk(qi, qp, xs, 2 * k + 1)
                    emit_av_chunk(qi, qp, xs, 2 * k)
                emit_divide(qi, qp, xs)
            for nt in range(8):
                emit_out_chunk(nb - 1, nt)
            pump(len(filler))

        if hw_loop:
            with tc.For_i(0, hw_loop):
                _batches()
        else:
            _batches()
    nc.compile()
    return nc


def host_prep(ft, w_qkv, b_qkv, w_out, b_out):
    ft = np.asarray(ft, dtype=np.float32)
    w_qkv = np.asarray(w_qkv, dtype=np.float32)
    b_qkv = np.asarray(b_qkv, dtype=np.float32)
    w_out = np.asarray(w_out, dtype=np.float32)
    b_out = np.asarray(b_out, dtype=np.float32)

    x = ft.reshape(B, N, C)
    xT = np.ascontiguousarray(x.transpose(0, 2, 1))

    w_qk_re = np.empty((C, 1024), np.float32)
    b_qk_re = np.empty((1024,), np.float32)
    w_v_re = np.empty((C, 512), np.float32)
    for p in range(4):
        hA, hB = 2 * p, 2 * p + 1
        w_qk_re[:, p * 128:p * 128 + 64] = w_qkv[:, hA * 192 + 64:hA * 192 + 128]
        w_qk_re[:, p * 128 + 64:p * 128 + 128] = w_qkv[:, hB * 192 + 64:hB * 192 + 128]
        b_qk_re[p * 128:p * 128 + 64] = b_qkv[hA * 192 + 64:hA * 192 + 128]
        b_qk_re[p * 128 + 64:p * 128 + 128] = b_qkv[hB * 192 + 64:hB * 192 + 128]
        w_qk_re[:, 512 + p * 128:512 + p * 128 + 64] = w_qkv[:, hA * 192:hA * 192 + 64]
        w_qk_re[:, 512 + p * 128 + 64:512 + p * 128 + 128] = w_qkv[:, hB * 192:hB * 192 + 64]
        b_qk_re[512 + p * 128:512 + p * 128 + 64] = b_qkv[hA * 192:hA * 192 + 64]
        b_qk_re[512 + p * 128 + 64:512 + p * 128 + 128] = b_qkv[hB * 192:hB * 192 + 64]
        w_v_re[:, p * 128:p * 128 + 64] = w_qkv[:, hA * 192 + 128:hA * 192 + 192]
        w_v_re[:, p * 128 + 64:p * 128 + 128] = w_qkv[:, hB * 192 + 128:hB * 192 + 192]

    b_v_nat = np.empty((512,), np.float32)
    for h in range(NH):
        b_v_nat[h * 64:(h + 1) * 64] = b_qkv[h * 192 + 128:h * 192 + 192]
    resid = x + b_out[None, None, :] + (b_v_nat @ w_out)[None, None, :]
    resid = np.ascontiguousarray(resid, dtype=np.float32)
    return xT, resid, w_qk_re, b_qk_re, w_v_re, np.ascontiguousarray(w_out)


_NC_CACHE = {}


def get_program(repeat: int = 1) -> bass.Bass:
    if repeat not in _NC_CACHE:
        _NC_CACHE[repeat] = build_program(repeat)
    return _NC_CACHE[repeat]


def make_in_maps(ft, w_qkv, b_qkv, w_out, b_out):
    xT, resid, w_qk_re, b_qk_re, w_v_re, w_out_c = host_prep(
        ft, w_qkv, b_qkv, w_out, b_out)
    in_maps = []
    for core in range(N_CORES):
        sl = slice(core * BL, (core + 1) * BL)
        in_maps.append({
            "xT": np.ascontiguousarray(xT[sl].astype(ml_dtypes.bfloat16)),
            "resid": np.ascontiguousarray(resid[sl]),
            "wqk": np.ascontiguousarray(w_qk_re.astype(ml_dtypes.bfloat16)),
            "bqk": b_qk_re,
            "wv": np.ascontiguousarray(w_v_re.astype(ml_dtypes.bfloat16)),
            "wout": np.ascontiguousarray(w_out_c.astype(ml_dtypes.bfloat16)),
        })
    return in_maps


_RUNNER_CACHE = {}


def make_runner(repeat: int = 1):
    """Build (once) a persistent jitted executor for the bass program.

    Returns run(in_maps) -> list[dict[name, np.ndarray]] per core. Keeping
    the jitted callable alive means repeat calls skip lowering/compile and
    are pure dispatch+execute.
    """
    if repeat in _RUNNER_CACHE:
        return _RUNNER_CACHE[repeat]

    import jax
    from jax.experimental.shard_map import shard_map
    from jax.sharding import Mesh, PartitionSpec
    from concourse import mybir as _mb
    from concourse import bass2jax

    bass2jax.install_neuronx_cc_hook()
    nc = get_program(repeat)

    partition_name = nc.partition_id_tensor.name if nc.partition_id_tensor else None
    in_names, out_names, out_avals, zero_shapes = [], [], [], []
    for alloc in nc.m.functions[0].allocations:
        if not isinstance(alloc, _mb.MemoryLocationSet):
            continue
        name = alloc.memorylocations[0].name
        if alloc.kind == "ExternalInput":
            if name != partition_name:
                in_names.append(name)
        elif alloc.kind == "ExternalOutput":
            np_dt = _mb.dt.np(alloc.dtype)
            out_names.append(name)
            out_avals.append(jax.core.ShapedArray(tuple(alloc.tensor_shape), np_dt))
            zero_shapes.append((tuple(alloc.tensor_shape), np_dt))
    n_params = len(in_names)
    all_in_names = list(in_names) + list(out_names)
    if partition_name is not None:
        all_in_names.append(partition_name)

    def _body(*args):
        operands = list(args)
        if partition_name is not None:
            operands.append(bass2jax.partition_id_tensor())
        outs = bass2jax._bass_exec_p.bind(
            *operands,
            out_avals=tuple(out_avals),
            in_names=tuple(all_in_names),
            out_names=tuple(out_names),
            lowering_input_output_aliases=(),
            sim_require_finite=True,
            sim_require_nnan=True,
            nc=nc,
        )
        return tuple(outs)

    devices = jax.devices()[:N_CORES]
    mesh = Mesh(np.asarray(devices), ("core",))
    n_outs = len(out_names)
    sharded = jax.jit(
        shard_map(_body, mesh=mesh,
                  in_specs=(PartitionSpec("core"),) * (n_params + n_outs),
                  out_specs=(PartitionSpec("core"),) * n_outs,
                  check_rep=False),
        keep_unused=True,
    )

    def run(in_maps):
        concat_in = [
            np.concatenate([np.asarray(m[name]) for m in in_maps], axis=0)
            for name in in_names
        ]
        zeros = [np.zeros((N_CORES * s[0], *s[1:]), dt) for s, dt in zero_shapes]
        out_arrs = sharded(*concat_in, *zeros)
        return [
            {name: np.asarray(out_arrs[i]).reshape(N_CORES, *out_avals[i].shape)[c]
             for i, name in enumerate(out_names)}
            for c in range(N_CORES)
        ]

    def make_chained(k):
        def _chain(*args):
            ins = list(args[:n_params])
            bufs = list(args[n_params:])
            for _ in range(k):
                bufs = list(_body(*ins, *bufs))
            return tuple(bufs)
        return jax.jit(
            shard_map(_chain, mesh=mesh,
                      in_specs=(PartitionSpec("core"),) * (n_params + n_outs),
                      out_specs=(PartitionSpec("core"),) * n_outs,
                      check_rep=False),
            keep_unused=True,
        )

    run.sharded = sharded
    run.in_names = in_names
    run.zero_shapes = zero_shapes
    run.make_chained = make_chained
    run.mesh = mesh
    _RUNNER_CACHE[repeat] = run
    return run


def kernel(ft, w_qkv, b_qkv, w_out, b_out):
    run = make_runner()
    in_maps = make_in_maps(ft, w_qkv, b_qkv, w_out, b_out)
    results = run(in_maps)
    out = np.concatenate([r["out"] for r in results], axis=0)
    return out.reshape(B, HH, WW, C).astype(np.float32)


# revision 50
# speedup vs baseline: 1.1754x; 1.0741x over previous
"""Trainium2 Bass kernel for nn_Attention_85564338471023.

Multi-head self-attention (B=16, N=1024 tokens, C=512, 8 heads x d=64) with
qkv projection, softmax attention, output projection and residual.

Sharding: pure data-parallel over batch -- 2 batch elements per NeuronCore,
no collectives. Host pre-transposes x (channels-on-partitions) and reorders
w_qkv columns so heads come in pairs that share 128-partition tiles.

Pipeline (204us baseline -> ~140us):
  * Head-pair concurrency on the PE: the two heads' score matmuls (K=64
    stationaries at row offsets 0/64) and AV/denominator matmuls (M=64
    outputs at partition offsets 0/64) are emitted adjacently so the PE
    row-/col-tiles them into disjoint quadrants and streams them
    concurrently. Score tiles must stay SEPARATE PSUM tiles per head --
    merging both heads into one [128,1024] tile serializes the pair.
  * PSUM (8 banks) split so all phases coexist: scores 5x[128,512]
    (5-slot pool decouples slot recycling from the exp drain order),
    res+s accumulators 2, projection scratch 1.
  * Single global software pipeline over the head-pair stream (across
    batch boundaries): pair g's scores+exp overlap pair g-1's AV
    matmuls; qkv/v projections of batch i+1 and the out-projection of
    batch i-1 are queued as filler chunks, one per step, placed BEFORE
    the score matmuls that wait on exp PSUM-slot releases so the
    in-order PE queue never stalls on ready work.
  * exp(x/8) on ScalarE with fused scale, no max-subtraction (scores are
    ~N(0,1)); softmax denominator via ones-stationary matmuls sharing
    the AV accumulation; 1/s via DVE reciprocal_approx_fast.
  * bf16 everywhere except the q/k tiles and score matmuls (f32r): x,
    w_qkv, w_v, w_out, AV operands and rt. All bf16 stationaries are
    64-col (split col-tiled halves for the projections) -- LDWEIGHTS
    fully hides under the streams and the 128-col fast-weight-load
    corruption path is never engaged. Final rel err ~3e-4.
"""

from collections import deque
from contextlib import ExitStack

import ml_dtypes
import numpy as np

import concourse.bacc as bacc
import concourse.bass as bass
import concourse.tile as tile
from concourse import mybir
from concourse.bass_utils import run_bass_kernel_spmd  # noqa: F401 (fallback path)

N_CORES = 8
B, HH, WW, C = 16, 32, 32, 512
N = HH * WW            # 1024 tokens
NH, DH = 8, 64
SCALE = DH ** -0.5     # 0.125
BL = B // N_CORES      # 2 batch elements per core
P = 128
F32 = mybir.dt.float32
F32R = mybir.dt.float32r
MM_DT = F32R
AV_DT = mybir.dt.bfloat16


def build_program(repeat=1) -> bass.Bass:
    inner, hw_loop = repeat if isinstance(repeat, tuple) else (repeat, None)
    nc = bacc.Bacc("TRN2", target_bir_lowering=False, debug=False)

    xT_d = nc.dram_tensor("xT", [BL, C, N], AV_DT, kind="ExternalInput")
    rs_d = nc.dram_tensor("resid", [BL, N, C], F32, kind="ExternalInput")
    wqk_d = nc.dram_tensor("wqk", [C, 1024], AV_DT, kind="ExternalInput")
    bqk_d = nc.dram_tensor("bqk", [1024], F32, kind="ExternalInput")
    wv_d = nc.dram_tensor("wv", [C, 512], AV_DT, kind="ExternalInput")
    wout_d = nc.dram_tensor("wout", [C, 512], AV_DT, kind="ExternalInput")
    out_d = nc.dram_tensor("out", [BL, N, C], F32, kind="ExternalOutput")

    with tile.TileContext(nc) as tc, ExitStack() as ctx:
        consts = ctx.enter_context(tc.tile_pool(name="consts", bufs=1))
        wpool = ctx.enter_context(tc.tile_pool(name="w", bufs=1))
        xt_pool = ctx.enter_context(tc.tile_pool(name="xt", bufs=8))
        qk_pool = ctx.enter_context(tc.tile_pool(name="qk", bufs=13))
        v_pool = ctx.enter_context(tc.tile_pool(name="v", bufs=16))
        ex_pool = ctx.enter_context(tc.tile_pool(name="ex", bufs=54))
        rt_pool = ctx.enter_context(tc.tile_pool(name="rt", bufs=22))
        misc = ctx.enter_context(tc.tile_pool(name="misc", bufs=2))
        # PSUM: 8 banks. scores 5x[128,512] (separate tiles per head and
        # x-half -- merging heads into one tile serializes the score
        # matmuls); res+s accumulators = 2; projections = 1.
        psb = ctx.enter_context(tc.tile_pool(name="psb", bufs=5, space="PSUM"))
        pst = ctx.enter_context(tc.tile_pool(name="pst", bufs=1, space="PSUM"))
        psp = ctx.enter_context(tc.tile_pool(name="psp", bufs=1, space="PSUM"))

        ones = consts.tile([P, 64], AV_DT, tag="ones")
        nc.vector.memset(ones[:], 1.0)
        bqk_sb = consts.tile([P, 8], F32, tag="bqk")
        nc.sync.dma_start(out=bqk_sb[:], in_=bqk_d.ap().rearrange("(t p) -> p t", p=P))

        wqk_sb, wv_sb, wout_sb = [], [], []
        for kc in range(4):
            t = wpool.tile([P, 1024], AV_DT, tag=f"wqk{kc}")
            nc.sync.dma_start(out=t[:], in_=wqk_d.ap()[kc * P:(kc + 1) * P, :])
            wqk_sb.append(t)
        for kc in range(4):
            t = wpool.tile([P, 512], AV_DT, tag=f"wv{kc}")
            nc.sync.dma_start(out=t[:], in_=wv_d.ap()[kc * P:(kc + 1) * P, :])
            wv_sb.append(t)
            t = wpool.tile([P, 512], AV_DT, tag=f"wout{kc}")
            nc.sync.dma_start(out=t[:], in_=wout_d.ap()[kc * P:(kc + 1) * P, :])
            wout_sb.append(t)

        # per-live-batch state, keyed by linear batch index i
        xt = {}      # i -> [4 tiles]
        qk = {}      # (i, jt) -> tile
        vo = {}      # (i, yt) -> tile
        ex = {}      # (i, p, yt, e, xs) -> tile
        rt = {}      # (i, p, xs) -> tile
        acc = {}     # (i, p, xs) -> (res_ps, s_ps)

        def emit_xt_dma(i):
            b = i % BL
            xt[i] = []
            for kc in range(4):
                t = xt_pool.tile([P, N], AV_DT)
                nc.sync.dma_start(out=t[:], in_=xT_d.ap()[b, kc * P:(kc + 1) * P, :])
                xt[i].append(t)

        def emit_qk_chunk(i, jt, xs):
            # bf16 stationaries split into 64-col halves: avoids the
            # 128-col fast-weight-load path, halves+hides LDWEIGHTS, and
            # the two halves run col-tiled concurrently.
            ps = psp.tile([P, 512], F32, tag="p")
            for kc in range(4):
                for e2 in range(2):
                    nc.tensor.matmul(
                        ps[e2 * 64:(e2 + 1) * 64, :],
                        wqk_sb[kc][:, jt * P + e2 * 64:jt * P + (e2 + 1) * 64],
                        xt[i][kc][:, xs * 512:(xs + 1) * 512],
                        start=(kc == 0), stop=(kc == 3), skip_group_check=True,
                    )
            if xs == 0:
                qk[(i, jt)] = qk_pool.tile([P, N], MM_DT, tag="qk",
                                           name=f"qk_{i}_{jt}")
            nc.vector.tensor_scalar(
                out=qk[(i, jt)][:, xs * 512:(xs + 1) * 512], in0=ps[:],
                scalar1=bqk_sb[:, jt:jt + 1], scalar2=None,
                op0=mybir.AluOpType.add,
            )

        def emit_v_chunk(i, yt):
            ps = psp.tile([P, 512], F32, tag="p")
            for kc in range(4):
                for e2 in range(2):
                    nc.tensor.matmul(
                        ps[e2 * 64:(e2 + 1) * 64, :],
                        xt[i][kc][:, yt * P + e2 * 64:yt * P + (e2 + 1) * 64],
                        wv_sb[kc][:],
                        start=(kc == 0), stop=(kc == 3), skip_group_check=True,
                    )
            t = v_pool.tile([P, 512], AV_DT)
            nc.vector.tensor_copy(t[:], ps[:])
            vo[(i, yt)] = t

        # Scores for one (pair, yt) step: 4 PSUM tiles [128,512] keyed
        # (e, xs), allocated in the order (e0,x0),(e1,x0),(e0,x1),(e1,x1)
        # to match the exp drain order; the matmul pairs (e0,x) + (e1,x)
        # are adjacent so the PE row-tiles them concurrently (K=64 heads
        # at row offsets 0 / 64).
        def emit_scores(i, p, yt):
            kk = qk[(i, p)]
            qq = qk[(i, 4 + p)]
            sc = {}
            for xs in range(2):
                for e in range(2):
                    sc[(e, xs)] = psb.tile([P, 512], F32, tag="big",
                                           name=f"sc_{e}_{xs}")
            for xs in range(2):
                for e in range(2):
                    nc.tensor.matmul(
                        sc[(e, xs)][:],
                        kk[e * 64:(e + 1) * 64, yt * P:(yt + 1) * P],
                        qq[e * 64:(e + 1) * 64, xs * 512:(xs + 1) * 512],
                        start=True, stop=True,
                    )
            for xs in range(2):
                for e in range(2):
                    t = ex_pool.tile([P, 512], AV_DT, tag="ex",
                                     name=f"ex_{e}_{xs}")
                    nc.scalar.activation(
                        out=t[:], in_=sc[(e, xs)][:],
                        func=mybir.ActivationFunctionType.Exp, scale=SCALE,
                    )
                    ex[(i, p, yt, e, xs)] = t

        # AV chunks for a (pair, xs) group are emitted in y-order
        # 1,0,3,2,5,4,7,6 (so the group-opening chunk never lands right
        # behind the divide that frees its PSUM slot): accumulation-group
        # flags go on the first/last EMITTED chunk.
        def emit_av_chunk(i, p, xs, yt):
            if yt == 1:
                res_ps = pst.tile([P, 512], F32, tag="res")
                s_ps = pst.tile([P, 512], F32, tag="s")
                acc[(i, p, xs)] = (res_ps, s_ps)
            res_ps, s_ps = acc[(i, p, xs)]
            first, last = yt == 1, yt == 6
            exs = [ex[(i, p, yt, e, xs)][:] for e in range(2)]

            def res_mms():
                for e in range(2):
                    h = 2 * p + e
                    nc.tensor.matmul(
                        res_ps[e * 64:(e + 1) * 64, :],
                        vo[(i, yt)][:, h * 64:(h + 1) * 64],
                        exs[e],
                        start=first, stop=last, skip_group_check=True,
                    )

            def s_mms():
                for e in range(2):
                    nc.tensor.matmul(
                        s_ps[e * 64:(e + 1) * 64, :],
                        ones[:],
                        exs[e],
                        start=first, stop=last, skip_group_check=True,
                    )

            # group-final chunk: s stops first so the divide's reciprocal
            # overlaps the trailing res-pair stream. group-opening chunk:
            # s first as well -- the previous divide's reciprocal releases
            # the s PSUM slot ~0.7us before the multiply releases res.
            if last or first:
                s_mms()
                res_mms()
            else:
                res_mms()
                s_mms()

        def emit_divide(i, p, xs):
            res_ps, s_ps = acc.pop((i, p, xs))
            rec = misc.tile([P, 512], F32, tag="prc")
            nc.vector.reciprocal_approx_fast(out=rec[:], in_=s_ps[:])
            t = rt_pool.tile([P, 512], AV_DT)
            nc.vector.tensor_tensor(
                out=t[:], in0=res_ps[:], in1=rec[:],
                op=mybir.AluOpType.mult,
            )
            rt[(i, p, xs)] = t
            if xs == 1:
                for yy in range(8):
                    for e in range(2):
                        for xx in range(2):
                            del ex[(i, p, yy, e, xx)]
                if p == 3:
                    for yy in range(8):
                        del vo[(i, yy)]

        def emit_out_chunk(i, nt):
            b = i % BL
            xs, sub = divmod(nt, 4)
            ps = psp.tile([P, 512], F32, tag="p")
            # rt stationaries split into 64-col halves: avoids the 128-col
            # bf16 fast-weight-load corruption and the two halves run
            # col-tiled concurrently (out partitions 0-63 / 64-127).
            for p in range(4):
                for e in range(2):
                    nc.tensor.matmul(
                        ps[e * 64:(e + 1) * 64, :],
                        rt[(i, p, xs)][:, sub * P + e * 64:sub * P + (e + 1) * 64],
                        wout_sb[p][:],
                        start=(p == 0), stop=(p == 3), skip_group_check=True,
                    )
            rs = misc.tile([P, 512], F32, tag="rs")
            nc.sync.dma_start(out=rs[:], in_=rs_d.ap()[b, nt * P:(nt + 1) * P, :])
            ob = misc.tile([P, 512], F32, tag="ob")
            nc.vector.tensor_tensor(
                out=ob[:], in0=ps[:], in1=rs[:], op=mybir.AluOpType.add,
            )
            nc.sync.dma_start(out=out_d.ap()[b, nt * P:(nt + 1) * P, :], in_=ob[:])
            if nt == 7:
                for p in range(4):
                    for xs2 in range(2):
                        del rt[(i, p, xs2)]

        def _batches():
            nb = inner * BL
            filler = deque()

            def pump(k):
                for _ in range(k):
                    if not filler:
                        return
                    filler.popleft()()

            # prologue: first batch's input + projections emitted directly
            emit_xt_dma(0)
            for jt in range(8):
                for xs in range(2):
                    emit_qk_chunk(0, jt, xs)
            for yt in range(8):
                emit_v_chunk(0, yt)
            if nb > 1:
                emit_xt_dma(1)

            def queue_batch_work(i):
                # Fillers consumed during batch i's 32 steps (1 per step):
                # projections for batch i+1 interleaved so qk tile slot
                # releases (one pair's kk+qq free every 8 steps) line up,
                # then the out-projection of batch i-1 (whose rt tiles
                # finish during this batch's pair-0 window).
                if i + 1 < nb:
                    if i + 2 < nb:
                        emit_xt_dma(i + 2)
                    def qkc(jt):
                        return [(lambda jt=jt, xs=xs: emit_qk_chunk(i + 1, jt, xs))
                                for xs in range(2)]
                    def vc(y0, y1):
                        return [(lambda yt=yt: emit_v_chunk(i + 1, yt))
                                for yt in range(y0, y1)]
                    for c in (qkc(0) + qkc(1) + qkc(2) + qkc(3) + vc(0, 4)
                              + qkc(4) + qkc(5) + vc(4, 8) + qkc(6) + qkc(7)):
                        filler.append(c)

            # Global pair stream: pair g's scores+exp overlap pair (g-1)'s
            # AV matmuls, across batch boundaries. Per-step PE order
            # [filler, AV, scores] keeps ready work ahead of the score
            # matmuls that wait on exp slot releases.
            G = 4 * nb
            for g in range(G):
                i, p = divmod(g, 4)
                if p == 0:
                    queue_batch_work(i)
                if p == 1 and i >= 1:
                    # batch i-1's rt tiles are complete once pair 0's AV
                    # divides have been emitted (during pair 0's steps)
                    filler.extend(
                        (lambda nt=nt: emit_out_chunk(i - 1, nt))
                        for nt in range(8))
                for yt in range(8):
                    xs, k = divmod(yt, 4)
                    if g >= 1 and k == 3:
                        # group-final step: AV + divide ahead of the filler
                        # so the reciprocal leads the DVE queue -- the next
                        # AV group's opening matmul waits on the PSUM slot
                        # this divide releases.
                        qi, qp = divmod(g - 1, 4)
                        emit_av_chunk(qi, qp, xs, 2 * k + 1)
                        emit_av_chunk(qi, qp, xs, 2 * k)
                        emit_divide(qi, qp, xs)
                        pump(1)
                    else:
                        pump(1)
                        if g >= 1:
                            qi, qp = divmod(g - 1, 4)
                            emit_av_chunk(qi, qp, xs, 2 * k + 1)
                            emit_av_chunk(qi, qp, xs, 2 * k)
                    emit_scores(i, p, yt)
            # epilogue: drain the last pair's AV + out-projection
            qi, qp = nb - 1, 3
            for xs in range(2):
                for k in range(4):
                    emit_av_chunk(qi, qp, xs, 2 * k + 1)
                    emit_av_chunk(qi, qp, xs, 2 * k)
                emit_divide(qi, qp, xs)
            for nt in range(8):
                emit_out_chunk(nb - 1, nt)
            pump(len(filler))

        if hw_loop:
            with tc.For_i(0, hw_loop):
                _batches()
        else:
            _batches()
    nc.compile()
    return nc


def host_prep(ft, w_qkv, b_qkv, w_out, b_out):
    ft = np.asarray(ft, dtype=np.float32)
    w_qkv = np.asarray(w_qkv, dtype=np.float32)
    b_qkv = np.asarray(b_qkv, dtype=np.float32)
    w_out = np.asarray(w_out, dtype=np.float32)
    b_out = np.asarray(b_out, dtype=np.float32)

    x = ft.reshape(B, N, C)
    xT = np.ascontiguousarray(x.transpose(0, 2, 1))

    w_qk_re = np.empty((C, 1024), np.float32)
    b_qk_re = np.empty((1024,), np.float32)
    w_v_re = np.empty((C, 512), np.float32)
    for p in range(4):
        hA, hB = 2 * p, 2 * p + 1
        w_qk_re[:, p * 128:p * 128 + 64] = w_qkv[:, hA * 192 + 64:hA * 192 + 128]
        w_qk_re[:, p * 128 + 64:p * 128 + 128] = w_qkv[:, hB * 192 + 64:hB * 192 + 128]
        b_qk_re[p * 128:p * 128 + 64] = b_qkv[hA * 192 + 64:hA * 192 + 128]
        b_qk_re[p * 128 + 64:p * 128 + 128] = b_qkv[hB * 192 + 64:hB * 192 + 128]
        w_qk_re[:, 512 + p * 128:512 + p * 128 + 64] = w_qkv[:, hA * 192:hA * 192 + 64]
        w_qk_re[:, 512 + p * 128 + 64:512 + p * 128 + 128] = w_qkv[:, hB * 192:hB * 192 + 64]
        b_qk_re[512 + p * 128:512 + p * 128 + 64] = b_qkv[hA * 192:hA * 192 + 64]
        b_qk_re[512 + p * 128 + 64:512 + p * 128 + 128] = b_qkv[hB * 192:hB * 192 + 64]
        w_v_re[:, p * 128:p * 128 + 64] = w_qkv[:, hA * 192 + 128:hA * 192 + 192]
        w_v_re[:, p * 128 + 64:p * 128 + 128] = w_qkv[:, hB * 192 + 128:hB * 192 + 192]

    b_v_nat = np.empty((512,), np.float32)
    for h in range(NH):
        b_v_nat[h * 64:(h + 1) * 64] = b_qkv[h * 192 + 128:h * 192 + 192]
    resid = x + b_out[None, None, :] + (b_v_nat @ w_out)[None, None, :]
    resid = np.ascontiguousarray(resid, dtype=np.float32)
    return xT, resid, w_qk_re, b_qk_re, w_v_re, np.ascontiguousarray(w_out)


_NC_CACHE = {}


def get_program(repeat: int = 1) -> bass.Bass:
    if repeat not in _NC_CACHE:
        _NC_CACHE[repeat] = build_program(repeat)
    return _NC_CACHE[repeat]


def make_in_maps(ft, w_qkv, b_qkv, w_out, b_out):
    xT, resid, w_qk_re, b_qk_re, w_v_re, w_out_c = host_prep(
        ft, w_qkv, b_qkv, w_out, b_out)
    in_maps = []
    for core in range(N_CORES):
        sl = slice(core * BL, (core + 1) * BL)
        in_maps.append({
            "xT": np.ascontiguousarray(xT[sl].astype(ml_dtypes.bfloat16)),
            "resid": np.ascontiguousarray(resid[sl]),
            "wqk": np.ascontiguousarray(w_qk_re.astype(ml_dtypes.bfloat16)),
            "bqk": b_qk_re,
            "wv": np.ascontiguousarray(w_v_re.astype(ml_dtypes.bfloat16)),
            "wout": np.ascontiguousarray(w_out_c.astype(ml_dtypes.bfloat16)),
        })
    return in_maps


_RUNNER_CACHE = {}


def make_runner(repeat: int = 1):
    """Build (once) a persistent jitted executor for the bass program.

    Returns run(in_maps) -> list[dict[name, np.ndarray]] per core. Keeping
    the jitted callable alive means repeat calls skip lowering/compile and
    are pure dispatch+execute.
    """
    if repeat in _RUNNER_CACHE:
        return _RUNNER_CACHE[repeat]

    import jax
    from jax.experimental.shard_map import shard_map
    from jax.sharding import Mesh, PartitionSpec
    from concourse import mybir as _mb
    from concourse import bass2jax

    bass2jax.install_neuronx_cc_hook()
    nc = get_program(repeat)

    partition_name = nc.partition_id_tensor.name if nc.partition_id_tensor else None
    in_names, out_names, out_avals, zero_shapes = [], [], [], []
    for alloc in nc.m.functions[0].allocations:
        if not isinstance(alloc, _mb.MemoryLocationSet):
            continue
        name = alloc.memorylocations[0].name
        if alloc.kind == "ExternalInput":
            if name != partition_name:
                in_names.append(name)
        elif alloc.kind == "ExternalOutput":
            np_dt = _mb.dt.np(alloc.dtype)
            out_names.append(name)
            out_avals.append(jax.core.ShapedArray(tuple(alloc.tensor_shape), np_dt))
            zero_shapes.append((tuple(alloc.tensor_shape), np_dt))
    n_params = len(in_names)
    all_in_names = list(in_names) + list(out_names)
    if partition_name is not None:
        all_in_names.append(partition_name)

    def _body(*args):
        operands = list(args)
        if partition_name is not None:
            operands.append(bass2jax.partition_id_tensor())
        outs = bass2jax._bass_exec_p.bind(
            *operands,
            out_avals=tuple(out_avals),
            in_names=tuple(all_in_names),
            out_names=tuple(out_names),
            lowering_input_output_aliases=(),
            sim_require_finite=True,
            sim_require_nnan=True,
            nc=nc,
        )
        return tuple(outs)

    devices = jax.devices()[:N_CORES]
    mesh = Mesh(np.asarray(devices), ("core",))
    n_outs = len(out_names)
    sharded = jax.jit(
        shard_map(_body, mesh=mesh,
                  in_specs=(PartitionSpec("core"),) * (n_params + n_outs),
                  out_specs=(PartitionSpec("core"),) * n_outs,
                  check_rep=False),
        keep_unused=True,
    )

    def run(in_maps):
        concat_in = [
            np.concatenate([np.asarray(m[name]) for m in in_maps], axis=0)
            for name in in_names
        ]
        zeros = [np.zeros((N_CORES * s[0], *s[1:]), dt) for s, dt in zero_shapes]
        out_arrs = sharded(*concat_in, *zeros)
        return [
            {name: np.asarray(out_arrs[i]).reshape(N_CORES, *out_avals[i].shape)[c]
             for i, name in enumerate(out_names)}
            for c in range(N_CORES)
        ]

    def make_chained(k):
        def _chain(*args):
            ins = list(args[:n_params])
            bufs = list(args[n_params:])
            for _ in range(k):
                bufs = list(_body(*ins, *bufs))
            return tuple(bufs)
        return jax.jit(
            shard_map(_chain, mesh=mesh,
                      in_specs=(PartitionSpec("core"),) * (n_params + n_outs),
                      out_specs=(PartitionSpec("core"),) * n_outs,
                      check_rep=False),
            keep_unused=True,
        )

    run.sharded = sharded
    run.in_names = in_names
    run.zero_shapes = zero_shapes
    run.make_chained = make_chained
    run.mesh = mesh
    _RUNNER_CACHE[repeat] = run
    return run


def kernel(ft, w_qkv, b_qkv, w_out, b_out):
    run = make_runner()
    in_maps = make_in_maps(ft, w_qkv, b_qkv, w_out, b_out)
    results = run(in_maps)
    out = np.concatenate([r["out"] for r in results], axis=0)
    return out.reshape(B, HH, WW, C).astype(np.float32)
